# revision 43
# baseline (speedup 1.0000x reference)
"""Trainium2 Bass kernel for CustomMultiHeadAttention.

Problem: x[2,2048,1024], 16 heads, Dh=64. y = MHA(x) with Q/K/V/O projections.

Sharding (8 cores, no collectives):
  core c -> batch b = c//4, head-quarter hq = c%4 (4 heads, 256 model cols).
  Each core computes Q,K,V projections for its 4 heads over the FULL batch
  sequence, attention for those heads, and a PARTIAL o_proj (its 256 rows of
  Wo). The host sums the 4 partials per batch and adds bo + bv@Wo (the
  "all-reduce after o_proj" done at unshard time; bv is folded into the final
  bias since softmax rows sum to 1).

On-chip dataflow (d-on-partitions for K/Q; the AV matmul is oriented so its
moving side is the short V|ones dim — out free = 65 instead of 512):
  xT   [1024, 2048]  (host-pretransposed, bf16)
  K^T  = lhsT=Wk_s[d_in,256], rhs=xT -> [256, 2048] bf16 (+bk per-partition)
  V    = lhsT=xT tile, rhs=Wv_s -> [2048, 256] bf16 (ones column appended)
  Q^T  like K^T, emitted per query tile -> [256, 2048] bf16 (+bq)
  S^T  per head = lhsT=K^T[64,k128], rhs=Q^T[64,q512]; head pairs row-packed
  P^T  = exp(S^T * 0.125) on ACT, batched 1024-wide over two key tiles
       (no max subtraction; |scores| <= ~9).
  O    per 128-query subtile = lhsT=P^T[k128,q128], rhs=[V|ones][k128,65]
       accumulated over key tiles -> [q128, 65] (col 64 = sumexp).
  O^T  = PE-transpose of (O * 1/sumexp) per head pair -> [128, q128] bf16,
       per-partition normalize is a free tensor_scalar on the PSUM read.
  y    = lhsT=O^T[256,q128], rhs=Wo_s[256,1024] -> partial [2048, 1024] f32
"""

import numpy as np
import ml_dtypes

import concourse.bass as bass
import concourse.mybir as mybir
import concourse.tile as tile
from concourse import bacc
from concourse.bass_utils import run_bass_kernel_spmd

P = 128
S = 2048
D = 1024
H = 16
DH = 64
HPC = 4          # heads per core
HD = HPC * DH    # 256 model cols per core
KO = D // P      # 8 contraction subtiles for the projections
KT_N = S // P    # 16 key tiles
QT = 512         # query tile (matmul free dim)
QT_N = S // QT   # 4 query tiles
SUB_N = QT // P  # 4 128-query subtiles per query tile
N_CORES = 8

BF16 = mybir.dt.bfloat16
F32 = mybir.dt.float32
I16 = mybir.dt.int16
EXP = mybir.ActivationFunctionType.Exp
MUL = mybir.AluOpType.mult
ADD = mybir.AluOpType.add

# Schraudolph fast-exp constants for the DVE-offloaded exp tiles:
# int16 bits = round(s * SCH_A + SCH_B) reinterpreted as bf16 give
# exp(s * 0.125) with ~1.5% piecewise-linear error (HW-verified: the DVE
# fp32->int16 convert rounds to nearest). The constant-factor part of the
# error cancels in the softmax normalization; only the ~1.4% spread lands
# on the output, scaled by sqrt(offloaded fraction).
SCH_A = 0.125 * 128.0 / float(np.log(2.0))
SCH_B = 16256.0 - 7.4
DVE_EXP_STEPS = (2, 5)

_CACHE = {}


def _build_program():
    nc = bacc.Bacc(
        "TRN2",
        target_bir_lowering=False,
        debug=False,
        enable_asserts=False,
        num_devices=N_CORES,
    )
    xT = nc.dram_tensor("xT", [D, S], BF16, kind="ExternalInput").ap()
    wq = nc.dram_tensor("wq", [D, HD], BF16, kind="ExternalInput").ap()
    wk = nc.dram_tensor("wk", [D, HD], BF16, kind="ExternalInput").ap()
    wv = nc.dram_tensor("wv", [D, HD], BF16, kind="ExternalInput").ap()
    wo = nc.dram_tensor("wo", [HD, D], BF16, kind="ExternalInput").ap()
    bq = nc.dram_tensor("bq", [HD], F32, kind="ExternalInput").ap()
    bk = nc.dram_tensor("bk", [HD], F32, kind="ExternalInput").ap()
    ident = nc.dram_tensor("ident", [P, P], BF16, kind="ExternalInput").ap()
    y = nc.dram_tensor("y", [S, D], BF16, kind="ExternalOutput").ap()

    with tile.TileContext(nc) as tc:
        _body(tc, y, xT, wq, wk, wv, wo, bq, bk, ident)
    nc.compile()
    return nc


def _body(tc, y, xT, wq, wk, wv, wo, bq, bk, ident):
    nc = tc.nc
    with (
        tc.tile_pool(name="const", bufs=1) as const,
        tc.tile_pool(name="big", bufs=1) as big,
        tc.tile_pool(name="work", bufs=3) as work,
        tc.tile_pool(name="exps", bufs=12) as exps,
        tc.tile_pool(name="yst", bufs=3) as yst,
        tc.tile_pool(name="psw", bufs=2, space="PSUM") as psw,  # wide [P,2,512]
        tc.tile_pool(name="psv", bufs=2, space="PSUM") as psv,  # AV out + transp
        tc.tile_pool(name="psy", bufs=1, space="PSUM") as psy,  # o_proj [P,512]
    ):
        # ---- constant / persistent tiles ----
        # DMA order is tuned so the first K-projection chunk (wk + xT cols
        # 0:512) and Q projection (wq) are ready as early as possible.
        wk_sb = const.tile([P, KO, HD], BF16, tag="wk")
        nc.sync.dma_start(wk_sb[:], wk.rearrange("(ko p) m -> p ko m", p=P))
        bk_sb = const.tile([P, HD // P], F32, tag="bk")
        nc.sync.dma_start(bk_sb[:], bk.rearrange("(o p) -> p o", p=P))

        xT_sb = big.tile([P, KO, S], BF16, tag="xT")
        xT_r = xT.rearrange("(ko p) s -> p ko s", p=P)

        def load_x(c):
            cs = slice(c * QT, (c + 1) * QT)
            nc.sync.dma_start(xT_sb[:, :, cs], xT_r[:, :, cs])

        load_x(0)
        wq_sb = const.tile([P, KO, HD], BF16, tag="wq")
        nc.sync.dma_start(wq_sb[:], wq.rearrange("(ko p) m -> p ko m", p=P))
        bq_sb = const.tile([P, HD // P], F32, tag="bq")
        nc.sync.dma_start(bq_sb[:], bq.rearrange("(o p) -> p o", p=P))
        load_x(1)
        wv_sb = const.tile([P, KO, HD], BF16, tag="wv")
        nc.sync.dma_start(wv_sb[:], wv.rearrange("(ko p) m -> p ko m", p=P))
        load_x(2)
        load_x(3)
        wo_sb = const.tile([P, HD // P, D], BF16, tag="wo")
        nc.sync.dma_start(wo_sb[:], wo.rearrange("(ks p) n -> p ks n", p=P))
        id_sb = const.tile([P, P], BF16, tag="ident")
        nc.sync.dma_start(id_sb[:], ident)

        # V with a trailing ones column: cols 0..63 = V, col 64 = 1, so the
        # AV matmul produces O rows with a sumexp column 64.
        v_sb = big.tile([P, KT_N, HPC, DH + 1], BF16, tag="v")
        nc.vector.memset(v_sb[:, :, :, DH : DH + 1], 1.0)

        qT_sb = big.tile([P, HD // P, S], BF16, tag="qT")
        kT_sb = big.tile([P, HD // P, S], BF16, tag="kT")
        oT_sb = big.tile([P, HD // P, S], BF16, tag="oT")

        # ---- PE warmup: dummy matmuls on a zeroed tile (no DMA dependency)
        # so the tensor engine is at full clock when the projections start ----
        wu = const.tile([P, QT], BF16, tag="wu")
        nc.vector.memset(wu[:], 0.0)
        pwu = psy.tile([P, QT], F32, tag="psy", name="pwu")
        for i in range(7):
            nc.tensor.matmul(
                pwu[:], lhsT=wu[:, 0:P], rhs=wu[:], start=True, stop=True
            )

        # Projection chunks run through the single-bank psy pool so the QK
        # psw ring stays exclusively ACT-paced. k/q chunks can be split into
        # two 4-matmul emission halves (the PSUM accumulation stays open
        # across interleaved QK steps — different banks, so the bank-wide
        # has_written clear of start=True matmuls is not an issue).
        def k_proj_c(mt, nt, lo=0, hi=KO, tile_box=None, c0=0, c1=QT):
            if lo == 0:
                tile_box[0] = psy.tile([P, QT], F32, tag="psy", name="pk")
            pk = tile_box[0]
            for ko in range(lo, hi):
                nc.tensor.matmul(
                    pk[:, c0:c1],
                    lhsT=wk_sb[:, ko, mt * P : (mt + 1) * P],
                    rhs=xT_sb[:, ko, nt * QT + c0 : nt * QT + c1],
                    start=(ko == 0),
                    stop=(ko == KO - 1),
                )
            if hi == KO:
                nc.vector.tensor_scalar_add(
                    kT_sb[:, mt, nt * QT + c0 : nt * QT + c1],
                    pk[:, c0:c1],
                    bk_sb[:, mt : mt + 1],
                )

        def k_parts(mt, nt):
            box = [None]
            return (
                lambda: k_proj_c(mt, nt, 0, KO // 2, box),
                lambda: k_proj_c(mt, nt, KO // 2, KO, box),
            )

        def v_proj(st):
            # V chunks pipeline through the po ring (2 banks, idle until the
            # first AV at P2's end): chunk N+1's matmuls overlap chunk N's
            # PSUM->SBUF copy, and the QK psw ring stays untouched so the
            # ACT stream is never stalled by V copy latency.
            pv = psv.tile([P, HD], F32, tag="po", name="pv")
            for ko in range(KO):
                nc.tensor.matmul(
                    pv[:],
                    lhsT=xT_sb[:, ko, st * P : (st + 1) * P],
                    rhs=wv_sb[:, ko, :],
                    start=(ko == 0),
                    stop=(ko == KO - 1),
                )
            nc.vector.tensor_copy(
                out=v_sb[:, st, :, 0:DH],
                in_=pv[:].rearrange("p (h c) -> p h c", h=HPC),
            )

        # ---- per query tile: Q^T projection, attention, partial o_proj ----
        def q_proj_c(qt, mt, pool=None):
            qs = slice(qt * QT, (qt + 1) * QT)
            if pool is None:
                pq = psy.tile([P, QT], F32, tag="psy", name="pq")
            else:
                pq = pool.tile([P, 2, QT], F32, tag="psw", name="pqw")[:, 0, :]
            for ko in range(KO):
                nc.tensor.matmul(
                    pq[:],
                    lhsT=wq_sb[:, ko, mt * P : (mt + 1) * P],
                    rhs=xT_sb[:, ko, qs],
                    start=(ko == 0),
                    stop=(ko == KO - 1),
                )
            nc.vector.tensor_scalar_add(
                qT_sb[:, mt, qs], pq[:], bq_sb[:, mt : mt + 1]
            )

        # Emission order biases the scheduler's priorities. Each phase is one
        # (qt, hp) QK+exp pass of 8 steps; `extras` maps a step index to
        # callables emitted right after that step, so AV chunks, o_proj
        # groups and projection chunks interleave into the QK stream. That
        # keeps the in-order PE queue supplied with ready work while the QK
        # matmuls pace themselves against ACT through the 2-deep psw ring.
        def alloc_exps():
            return [
                [
                    exps.tile(
                        [P, KT_N // 2, QT], BF16, tag="exps", name=f"exp_{h2i}{ha}"
                    )
                    for ha in range(2)
                ]
                for h2i in range(2)
            ]

        def qk_phase(qt, hp, extras=None, pre=(), exp_t=None):
            qs = slice(qt * QT, (qt + 1) * QT)
            if exp_t is None:
                exp_t = alloc_exps()
            # `pre` work is emitted after this phase's exp-ring slots are
            # claimed but before the first QK write. Readers of the previous
            # ring generation placed here stay safe because both they and
            # the QK writes run in-order on PE.
            for fn in pre:
                fn()
            for step in range(KT_N // 2):
                kt = 2 * step
                half, lkt = step // 4, kt % (KT_N // 2)
                for h2 in range(2):
                    pr = slice(h2 * DH, (h2 + 1) * DH)
                    pqk = psw.tile([P, 2, QT], F32, tag="psw", name="pqk")
                    for j in range(2):
                        nc.tensor.matmul(
                            pqk[:, j, :],
                            lhsT=kT_sb[pr, hp, (kt + j) * P : (kt + j + 1) * P],
                            rhs=qT_sb[pr, hp, qs],
                            start=True,
                            stop=True,
                        )
                    if step in DVE_EXP_STEPS:
                        nc.vector.tensor_scalar(
                            exp_t[h2][half][:, lkt : lkt + 2, :].bitcast(I16),
                            pqk[:],
                            SCH_A,
                            SCH_B,
                            MUL,
                            ADD,
                        )
                    else:
                        nc.scalar.activation(
                            exp_t[h2][half][:, lkt : lkt + 2, :],
                            pqk[:],
                            EXP,
                            scale=0.125,
                        )
                if extras:
                    for fn in extras.get(step, ()):
                        fn()
            return exp_t

        # AV with the short dim moving: per 128-query subtile, the exp chunk
        # [k128, q128] is the stationary operand and [V|ones] [k128, 65] the
        # moving one, so each matmul streams only 65 columns. The sumexp in
        # col 64 makes normalization a per-partition tensor_scalar, and the
        # head-pair result is PE-transposed back to d-on-partitions for
        # o_proj.
        def av_sub(qt, hp, exp_t, sub, ob, rb, pool=None):
            q0 = sub * P
            # One PSUM bank per h2 chain: a start=True matmul clears
            # has_written for its whole bank, so concurrent accumulation
            # chains must not share a bank. The po tile is claimed just
            # before each h2 chain so the previous chain's ring slot
            # (released by its recip/norm on DVE) is waited on while THIS
            # chain's matmuls still have the other slot to run in.
            for h2 in range(2):
                h = 2 * hp + h2
                po = psv.tile([P, 72], F32, tag="po", name=f"po{h2}")
                for kt in range(KT_N):
                    nc.tensor.matmul(
                        po[:, 0 : DH + 1],
                        lhsT=exp_t[h2][kt // (KT_N // 2)][
                            :, kt % (KT_N // 2), q0 : q0 + P
                        ],
                        rhs=v_sb[:, kt, h, :],
                        start=(kt == 0),
                        stop=(kt == KT_N - 1),
                    )
                nc.vector.reciprocal(
                    rb[:, sub, h2 : h2 + 1], po[:, DH : DH + 1]
                )
                nc.vector.tensor_scalar_mul(
                    ob[:, sub, h2, :],
                    po[:, 0:DH],
                    rb[:, sub, h2 : h2 + 1],
                )
            tr = psv.tile([P, P], BF16, tag="tr", name="tr", bufs=1)
            nc.tensor.transpose(
                tr[:], ob[:, sub, :, :].rearrange("p a b -> p (a b)"), id_sb[:]
            )
            nc.vector.tensor_copy(
                out=oT_sb[:, hp, qt * QT + q0 : qt * QT + q0 + P], in_=tr[:]
            )

        def make_av(qt, hp, exp_t, pool=None):
            ob = work.tile([P, SUB_N, 2, DH], BF16, tag="ob", name="ob")
            rb = work.tile([P, SUB_N, 2], F32, tag="rb", name="rb")
            return [
                (lambda s=s: av_sub(qt, hp, exp_t, s, ob, rb, pool))
                for s in range(SUB_N)
            ]

        def o_proj_group(qt, g, ytiles, pool=None):
            st, nt2 = g // 2, g % 2
            if nt2 == 0:
                ytiles[st] = yst.tile([P, D], BF16, tag="yt", name="yt")
            yt = ytiles[st]
            rows = slice(qt * QT + st * P, qt * QT + (st + 1) * P)
            if pool is None:
                py = psy.tile([P, QT], F32, tag="psy", name="py")
            else:
                # tail only: the QK psw banks are free by then, so borrow a
                # 2-bank slot for a deeper o_proj ring
                py = pool.tile([P, 2, QT], F32, tag="psw", name="pyt")[:, 0, :]
            for ks in range(HD // P):
                nc.tensor.matmul(
                    py[:],
                    lhsT=oT_sb[:, ks, rows],
                    rhs=wo_sb[:, ks, nt2 * QT : (nt2 + 1) * QT],
                    start=(ks == 0),
                    stop=(ks == HD // P - 1),
                )
            nc.vector.tensor_copy(
                out=yt[:, nt2 * QT : (nt2 + 1) * QT], in_=py[:]
            )
            nc.sync.dma_start(
                y[rows, nt2 * QT : (nt2 + 1) * QT],
                yt[:, nt2 * QT : (nt2 + 1) * QT],
            )

        def make_op(qt, pool=None):
            ytiles = {}
            return [
                (lambda g=g: o_proj_group(qt, g, ytiles, pool))
                for g in range(2 * SUB_N)
            ]

        # Software-pipelined schedule. 8 QK phases; K/Q/V projection chunks
        # fill the front phases (k-chunks split into two 4-matmul halves so
        # PE work stays fine-grained against the ACT pace), then each phase
        # carries the AV of the pair whose exps ACT finished ~a phase ago
        # plus the o_proj of the pair before that. The 12-deep exps ring
        # holds 3 phases in flight, which pins av(p) to phase p+2 at the
        # latest.
        vp = lambda s: (lambda: v_proj(s))
        qc = lambda qt, mt: (lambda: q_proj_c(qt, mt))

        # pre-phase critical path: the first QK step needs only kT cols
        # 0:256 and qT(qt0, hp0); k000 runs through psy while q00 borrows a
        # psw slot so their PSUM accumulations overlap
        k_proj_c(0, 0, tile_box=[None], c0=0, c1=P * 2)
        q_proj_c(0, 0, pool=psw)
        k001 = lambda: k_proj_c(0, 0, tile_box=[None], c0=P * 2, c1=QT)
        k01a, k01b = k_parts(0, 1)
        k02a, k02b = k_parts(0, 2)
        k03a, k03b = k_parts(0, 3)
        k10a, k10b = k_parts(1, 0)
        k11a, k11b = k_parts(1, 1)
        k12a, k12b = k_parts(1, 2)
        k13a, k13b = k_parts(1, 3)
        e00 = qk_phase(0, 0, {
            0: [k001, k01a], 1: [k01b], 2: [k02a], 3: [k02b],
            4: [k03a], 5: [k03b], 6: [k10a], 7: [k10b, qc(0, 1)],
        })
        e01 = qk_phase(0, 1, {
            0: [k11a], 1: [k11b], 2: [k12a], 3: [k12b],
            4: [k13a], 5: [k13b], 6: [qc(1, 0), vp(0)], 7: [qc(1, 1), vp(1)],
        })
        av00 = make_av(0, 0, e00)
        e10 = qk_phase(1, 0, {
            0: [vp(2), vp(3)], 1: [vp(4), vp(5)], 2: [vp(6), vp(7)],
            3: [vp(8), vp(9)], 4: [vp(10), vp(11)], 5: [vp(12), vp(13)],
            6: [vp(14), vp(15), av00[0]], 7: [av00[1], av00[2], av00[3]],
        })
        av01 = make_av(0, 1, e01)
        op0 = make_op(0)
        e11 = qk_phase(1, 1, {
            0: [av01[0]], 1: [av01[1]], 2: [av01[2]], 3: [av01[3]],
            4: [op0[0], op0[1]], 5: [op0[2], op0[3], qc(2, 0)],
            6: [op0[4], op0[5]], 7: [op0[6], op0[7], qc(2, 1)],
        })
        av10 = make_av(1, 0, e10)
        e20 = qk_phase(2, 0, {
            0: [qc(3, 0)],
            2: [av10[0]], 3: [av10[1]], 4: [av10[2]], 5: [av10[3]],
            6: [qc(3, 1)],
        })
        av11 = make_av(1, 1, e11)
        op1 = make_op(1)
        e21 = qk_phase(2, 1, {
            0: [av11[0]], 1: [av11[1]], 2: [av11[2]], 3: [av11[3]],
            4: [op1[0], op1[1]], 5: [op1[2], op1[3]],
            6: [op1[4], op1[5]], 7: [op1[6], op1[7]],
        })
        av20 = make_av(2, 0, e20)
        av21 = make_av(2, 1, e21)
        e30 = qk_phase(3, 0, {
            1: [av20[0]], 2: [av20[1]], 3: [av20[2]], 4: [av20[3]],
            5: [av21[0]], 6: [av21[1]], 7: [av21[2]],
        })
        op2 = make_op(2)
        av30 = make_av(3, 0, e30)
        e31_t = alloc_exps()
        av31 = make_av(3, 1, e31_t)
        qk_phase(3, 1, {
            0: [av21[3]], 1: [op2[0], op2[1]], 2: [op2[2], op2[3]],
            3: [op2[4], op2[5]], 4: [op2[6], op2[7]],
            5: [av30[0]], 6: [av30[1], av30[2]], 7: [av30[3], av31[0]],
        }, exp_t=e31_t)
        op3 = make_op(3, pool=psw)
        tail = [
            op3[0], av31[1], op3[1], av31[2], op3[2], op3[3],
            av31[3], op3[4], op3[5], op3[6], op3[7],
        ]
        for fn in tail:
            fn()


def _prep_inputs(x, Wq, bq, Wk, bk, Wv, bv, Wo, bo):
    bf = ml_dtypes.bfloat16
    x = np.asarray(x, np.float32)
    ident = np.eye(P, dtype=bf)
    in_maps = []
    for c in range(N_CORES):
        b, hq = c // 4, c % 4
        cs = slice(hq * HD, (hq + 1) * HD)
        in_maps.append(
            {
                "xT": np.ascontiguousarray(x[b].T).astype(bf),
                "wq": np.ascontiguousarray(np.asarray(Wq, np.float32)[:, cs]).astype(bf),
                "wk": np.ascontiguousarray(np.asarray(Wk, np.float32)[:, cs]).astype(bf),
                "wv": np.ascontiguousarray(np.asarray(Wv, np.float32)[:, cs]).astype(bf),
                "wo": np.ascontiguousarray(np.asarray(Wo, np.float32)[cs, :]).astype(bf),
                "bq": np.ascontiguousarray(np.asarray(bq, np.float32)[cs]),
                "bk": np.ascontiguousarray(np.asarray(bk, np.float32)[cs]),
                "ident": ident,
            }
        )
    return in_maps


def get_program():
    if "nc" not in _CACHE:
        _CACHE["nc"] = _build_program()
    return _CACHE["nc"]


def run(inputs, **kw):
    nc = get_program()
    in_maps = _prep_inputs(**inputs)
    res = run_bass_kernel_spmd(nc, in_maps, core_ids=list(range(N_CORES)), **kw)
    # final bias: bo + bv @ Wo (bv folds out of attention since softmax rows
    # sum to 1), computed in fp32 on host
    bias = np.asarray(inputs["bo"], np.float32) + np.asarray(
        inputs["bv"], np.float32
    ) @ np.asarray(inputs["Wo"], np.float32)
    out = np.empty((2, S, D), np.float32)
    for b in range(2):
        acc = np.asarray(res.results[4 * b]["y"], np.float32).copy()
        for i in range(1, 4):
            acc += res.results[4 * b + i]["y"]
        out[b] = acc + bias
    return out, res


def kernel(**inputs):
    out, _ = run(inputs)
    return out
